# revision 1
# baseline (speedup 1.0000x reference)
"""Trainium2 Bass kernel for dense causal(-penalty) attention.

Problem: x[4,4096,512], Wq/Wk/Wv[512,64] -> out[4,4096,64]
  q,k,v = x@W;  scores = (1/8) q k^T;  masked = scores - 20*strict_upper;
  out = softmax(masked) @ v

Sharding: 8 cores = (batch b in 0..3) x (query half h in 0..1).
Each core computes a [2048, 64] output block. Inputs are passed per-core with
the batch's sequence permuted so the core's q-block comes first; the causal
mask then becomes (a) a local triangular mask on k-chunks 0..15 and (b) a
constant additive bias (0 or -20) on k-chunks 16..31, passed per-core.

On-device algorithm (flash-style, transposed scores, fp32r matmuls):
  KT[64,4096], QT[64,2048] (both duplicated onto partitions 64..127),
  V[4096,65] (65th col = ones, giving the softmax denominator for free).
  Loop over (k-chunk pair p, 512-wide q window w):
     ST[:,0:512]   = KT_{2p}^T   @ QT_w   (PE row-group 0..63)
     ST[:,512:1024]= KT_{2p+1}^T @ QT_w   (PE row-group 64..127, concurrent)
     PT = exp(ST/8 + bias)                (one ACT call per pair)
     boundary pairs: multiply by triangular patterns (DVE)
     OT_w += V_aug^T @ PT                 (PSUM accumulate per window)
  epilogue: transpose OT -> [2048, 65], divide by ones-column, DMA out.
The emission is software-pipelined so PE never stalls on ACT.
"""
import math
import numpy as np

import concourse.bass as bass
import concourse.mybir as mybir
import concourse.tile as tile
from concourse import bacc
from concourse.bass_utils import run_bass_kernel_spmd
from concourse.masks import make_identity

F32 = mybir.dt.float32
F32R = mybir.dt.float32r
BF16 = mybir.dt.bfloat16
AF = mybir.ActivationFunctionType

B, S, D, H = 4, 4096, 512, 64
Q = S // 2            # per-core q block
QW = 512              # q window per pipeline step
NCHUNK = D // 128     # 4 contraction chunks
NKC = S // 128        # 32 k-chunks
NPAIR = NKC // 2      # 16 k-chunk pairs
NEG = -20.0
EPS = math.exp(NEG)
SCALE = 0.125

_CACHE = {}


def _build(repeat=1):
    nc = bacc.Bacc("TRN2", target_bir_lowering=False, debug=False, num_devices=8)
    xt_d = nc.dram_tensor("xt", [D, S], F32R, kind="ExternalInput").ap()
    wq_d = nc.dram_tensor("wq", [D, H], F32R, kind="ExternalInput").ap()
    wk_d = nc.dram_tensor("wk", [D, H], F32R, kind="ExternalInput").ap()
    wv_d = nc.dram_tensor("wv", [D, H], F32R, kind="ExternalInput").ap()
    cb_d = nc.dram_tensor("cb", [128, 1], F32, kind="ExternalInput").ap()
    out_d = nc.dram_tensor("out", [Q, H], F32, kind="ExternalOutput").ap()

    with tile.TileContext(nc) as tc:
        with tc.tile_pool(name="big", bufs=1) as big, \
             tc.tile_pool(name="cst", bufs=1) as cst, \
             tc.tile_pool(name="pt", bufs=3) as ptp, \
             tc.tile_pool(name="osb", bufs=1) as osb, \
             tc.tile_pool(name="stp", bufs=2, space="PSUM") as stp, \
             tc.tile_pool(name="otp", bufs=2, space="PSUM") as otp:
          for _rep in range(repeat):
            # ---- constants ----
            cbias = cst.tile([128, 1], F32, tag="cb")
            nc.sync.dma_start(cbias[:], cb_d)
            neg20 = cst.tile([128, 1], F32, tag="n20")
            nc.gpsimd.memset(neg20[:], NEG)
            ones32 = cst.tile([128, NKC], F32, tag="ones")
            nc.gpsimd.memset(ones32[:], 1.0)
            ident = cst.tile([128, 128], F32, tag="id")
            # master triangular pattern: MA[kk, t] = 1 if t >= 384 + kk
            # else exp(-20); dpat[r] == MA[:, 384-128r : 896-128r]
            dmast = cst.tile([128, QW + 384], F32R, tag="dmast")

            def emit_constants():
                make_identity(nc, ident[:])
                d32 = cst.tile([128, QW + 384], F32, tag="dm32")
                nc.gpsimd.memset(d32[:], 1.0)
                nc.gpsimd.affine_select(
                    out=d32[:], in_=d32[:],
                    compare_op=mybir.AluOpType.is_ge,
                    fill=EPS, base=-384,
                    pattern=[[1, QW + 384]], channel_multiplier=-1,
                )
                nc.vector.tensor_copy(dmast[:], d32[:])

            def dpat_slice(r):
                return dmast[:, 384 - 128 * r:384 - 128 * r + QW]

            # ---- weights first (HWDGE drains in emission order), then x^T in
            # column-sliced pieces so the first projections start early ----
            ws = {}

            def load_w(nm, dram):
                t = cst.tile([128, NCHUNK * H], F32R, tag=nm, name=nm)
                nc.sync.dma_start(
                    t[:].rearrange("p (c h) -> p c h", h=H),
                    dram.rearrange("(c p) h -> p c h", p=128))
                ws[nm] = t

            load_w("wq", wq_d)
            load_w("wk", wk_d)
            xt = [big.tile([128, S], F32R, tag=f"xt{dc}", name=f"xt{dc}")
                  for dc in range(NCHUNK)]
            slices = [(0, 512), (512, 512)] + \
                     [(1024 * cs, 1024) for cs in (1, 2, 3)]
            for si, (lo, ln) in enumerate(slices):
                for dc in range(NCHUNK):
                    nc.sync.dma_start(
                        xt[dc][:, lo:lo + ln],
                        xt_d[128 * dc:128 * dc + 128, lo:lo + ln])
                if si == 0:
                    # wv is first needed by v_group(0), three projection
                    # groups after the first x^T slice lands
                    load_w("wv", wv_d)

            # ---- projections: QT/KT computed on partitions 0..63, then
            # duplicated onto 64..127 via SBUF->SBUF SWDGE DMA so the packed
            # ST matmuls can feed both PE row-groups. V is computed s-major
            # directly (x^T chunk stationary, Wv moving). ----
            qtd = big.tile([128, Q], F32R, tag="qtd")
            ktd = big.tile([128, S], F32R, tag="ktd")
            vsb = big.tile([128, NKC * (H + 1)], F32R, tag="v")
            nc.vector.tensor_copy(vsb[:, H::H + 1], ones32[:])

            proj_pend = {}

            def proj_half(ns, names, first):
                # emit half of a projection accumulation group (chunks 0-1 or
                # 2-3); the second half adds the copies + duplication
                if first:
                    ps = stp.tile([128 if len(names) == 2 else H, 512], F32,
                                  tag="vps", name="psproj")
                    proj_pend[(ns, tuple(names))] = ps
                    dcs = range(0, NCHUNK // 2)
                else:
                    ps = proj_pend.pop((ns, tuple(names)))
                    dcs = range(NCHUNK // 2, NCHUNK)
                for dc in dcs:
                    for gi, nm in enumerate(names):
                        nc.tensor.matmul(
                            ps[H * gi:H * gi + H, :],
                            ws[nm][:, H * dc:H * dc + H],
                            xt[dc][:, 512 * ns:512 * ns + 512],
                            start=(dc == 0 and gi == 0),
                            stop=(dc == NCHUNK - 1 and gi == len(names) - 1))
                if first:
                    return
                for gi, nm in enumerate(names):
                    sl = slice(512 * ns, 512 * ns + 512)
                    half = slice(H * gi, H * gi + H)
                    if nm == "wq":
                        nc.vector.tensor_copy(qtd[half, sl], ps[half, :])
                        nc.gpsimd.dma_start(
                            qtd[H - H * gi:2 * H - H * gi, sl], qtd[half, sl])
                    else:
                        nc.vector.tensor_copy(ktd[half, sl], ps[half, :])
                        nc.gpsimd.dma_start(
                            ktd[H - H * gi:2 * H - H * gi, sl], ktd[half, sl])

            def proj_group(ns, names):
                proj_half(ns, names, True)
                proj_half(ns, names, False)

            def v_group(st):
                ps = stp.tile([128, H], F32, tag="vps", name="psv")
                for dc in range(NCHUNK):
                    nc.tensor.matmul(
                        ps[:],
                        xt[dc][:, 128 * st:128 * st + 128],
                        ws["wv"][:, H * dc:H * dc + H],
                        start=(dc == 0), stop=(dc == NCHUNK - 1))
                nc.vector.tensor_copy(
                    vsb[:, (H + 1) * st:(H + 1) * st + H], ps[:])

            otsb = osb.tile([H + 1, Q], F32, tag="ot")
            outsb = osb.tile([128, (Q // 128) * H], F32, tag="outsb")

            # iteration m -> (pair p, window w); windows processed in groups
            # of two so each pair's V chunks are produced once per two steps
            def pw(m):
                g, r = divmod(m, 2 * NPAIR)
                p, wi = divmod(r, 2)
                return p, 2 * g + wi

            NIT = 2 * 2 * NPAIR  # 64
            ot_of = {}

            def emit_st(m):
                p, w = pw(m)
                if p == 0:
                    ot_of[w] = otp.tile([H + 1, QW], F32, tag="otps",
                                        name=f"otps{w}")
                stt = stp.tile([128, 2 * QW], F32, tag="st", name="stt")
                # First iterations run unpacked on row-group 0 so they don't
                # wait for the partition-duplication DMAs of ktd/qtd.
                dup_ready = m >= 4
                for half in range(2):
                    kc = 2 * p + half
                    hb = H * half if dup_ready else 0
                    nc.tensor.matmul(
                        stt[:, QW * half:QW * half + QW],
                        ktd[hb:hb + H, 128 * kc:128 * kc + 128],
                        qtd[hb:hb + H, QW * w:QW * w + QW],
                        start=True, stop=True)
                return stt

            def emit_exp(m, stt):
                p, w = pw(m)
                pt = ptp.tile([128, 2 * QW], F32R, tag="pt", name="ptt")
                if p >= 8:
                    bias = cbias[:]
                elif p <= 2 * w - 1:
                    bias = 0.0
                elif p >= 2 * w + 2:
                    bias = neg20[:]
                else:
                    bias = 0.0
                nc.scalar.activation(pt[:], stt[:], AF.Exp,
                                     bias=bias, scale=SCALE)
                if p < 8 and 2 * w <= p <= 2 * w + 1:
                    for half in range(2):
                        r = 2 * p + half - 4 * w
                        if 0 <= r < 4:
                            nc.vector.tensor_mul(
                                pt[:, QW * half:QW * half + QW],
                                pt[:, QW * half:QW * half + QW],
                                dpat_slice(r))
                        elif r >= 4:  # fully masked half
                            nc.vector.tensor_scalar_mul(
                                pt[:, QW * half:QW * half + QW],
                                pt[:, QW * half:QW * half + QW], EPS)
                return pt

            def emit_pv(m, pt):
                p, w = pw(m)
                otps = ot_of[w]
                for half in range(2):
                    kc = 2 * p + half
                    nc.tensor.matmul(
                        otps[:],
                        vsb[:, (H + 1) * kc:(H + 1) * (kc + 1)],
                        pt[:, QW * half:QW * half + QW],
                        start=(p == 0 and half == 0),
                        stop=(p == NPAIR - 1 and half == 1))
                if p == NPAIR - 1:
                    nc.vector.tensor_copy(
                        otsb[:, QW * w:QW * w + QW], otps[:])

            def emit_out(qt_i):
                tp = stp.tile([128, H + 1], F32, tag="vps", name="tp")
                nc.tensor.transpose(
                    tp[:], otsb[:, 128 * qt_i:128 * qt_i + 128],
                    ident[0:H + 1, 0:H + 1])
                rc = ptp.tile([128, 1], F32, tag="rc")
                nc.vector.reciprocal(rc[:], tp[:, H:H + 1])
                nc.vector.tensor_scalar_mul(
                    outsb[:, H * qt_i:H * qt_i + H], tp[:, 0:H], rc[:])

            # ---- software-pipelined emission ----
            # projection plan: ns0,1 carry (wq+wk) col-packed; ns2..7 carry
            # (wk+wv) col-packed (VT rows 64:128); wv ns0,1 and wq ns2,3 alone
            proj_group(0, ["wq"])
            proj_group(1, ["wq"])
            proj_group(0, ["wk"])
            emit_constants()
            v_group(0)
            v_group(1)
            st_cur = emit_st(0)
            pt_prev = None
            for m in range(NIT):
                p, w = pw(m)
                st_next = emit_st(m + 1) if m + 1 < NIT else None
                pt_cur = emit_exp(m, st_cur)
                # interleave the remaining projection groups
                if m % 4 == 0 and 1 + m // 4 <= 7:
                    proj_group(1 + m // 4, ["wk"])
                if m == 9:
                    proj_group(2, ["wq"])
                elif m == 13:
                    proj_group(3, ["wq"])
                # V chunks m+2, m+3 (first used at PV of m+2)
                if m % 2 == 0 and m + 3 < NKC:
                    v_group(m + 2)
                    v_group(m + 3)
                if pt_prev is not None:
                    emit_pv(m - 1, pt_prev)
                # spread the first window group's epilogue into the loop,
                # and ship its half of the output while the loop still runs
                if 36 <= m < 36 + Q // 256:
                    emit_out(m - 36)
                if m == 36 + Q // 256:
                    nc.sync.dma_start(
                        out_d[0:Q // 2].rearrange("(t p) h -> p t h", p=128),
                        outsb[:, 0:(Q // 256) * H]
                        .rearrange("p (t h) -> p t h", h=H))
                st_cur, pt_prev = st_next, pt_cur
            # window-2 output tiles are ready (flushed at the last loop step);
            # their transposes fill PE's wait on the final exp
            for qt_i in range(Q // 256, 3 * Q // 512):
                emit_out(qt_i)
            emit_pv(NIT - 1, pt_prev)
            for qt_i in range(3 * Q // 512, Q // 128):
                emit_out(qt_i)
            nc.sync.dma_start(
                out_d[Q // 2:Q].rearrange("(t p) h -> p t h", p=128),
                outsb[:, (Q // 256) * H:]
                .rearrange("p (t h) -> p t h", h=H))
    nc.compile()
    return nc


def kernel(x, Wq, Wk, Wv):
    x = np.ascontiguousarray(np.asarray(x, dtype=np.float32))
    Wq = np.asarray(Wq, dtype=np.float32)
    Wk = np.asarray(Wk, dtype=np.float32)
    Wv = np.asarray(Wv, dtype=np.float32)

    if "nc" not in _CACHE:
        _CACHE["nc"] = _build()
    nc = _CACHE["nc"]

    in_maps = []
    for c in range(8):
        b, h = c // 2, c % 2
        xp = np.concatenate(
            [x[b, Q * h:Q * h + Q], x[b, Q * (1 - h):Q * (1 - h) + Q]], axis=0)
        cb = np.full((128, 1), 0.0 if h == 1 else NEG, dtype=np.float32)
        in_maps.append({
            "xt": np.ascontiguousarray(xp.T),
            "wq": Wq, "wk": Wk, "wv": Wv, "cb": cb,
        })

    res = run_bass_kernel_spmd(nc, in_maps, list(range(8)))

    out = np.empty((B, S, H), dtype=np.float32)
    for c in range(8):
        b, h = c // 2, c % 2
        out[b, Q * h:Q * h + Q] = res.results[c]["out"]
    return out

